# revision 9
# baseline (speedup 1.0000x reference)
"""Trainium2 Bass kernel for nn_Attention (B=2, L=2048, DIM=1024, H=16, D=64).

Sharding: 8 cores, each handles one (b, 4-head-group) pair — data parallel
on B (cores 0-3 -> b=0, cores 4-7 -> b=1), tensor parallel on heads
(4 heads per core). The output projection is computed per-core over the
core's 4 heads; the host sums the 4 partials per batch and adds the bias.

v3: the exp (ACT engine) is the per-core roofline (~128us busy), and the
PE's tiling-mode switches were the hidden tax (measured ~340ns drain per
64-row-mode <-> 128-mode transition in the S<->AV alternation). Design:
  - K^T is stored zero-padded per head ([128 partitions] = head's 64 d
    rows in its half, zeros in the other half) so the S matmul is a
    plain 128-contraction matmul against the packed Q tile (the zero
    rows annihilate the other head's Q half). EVERY matmul in the
    kernel is then 128-contraction / 128x128-mode: no mode switches.
  - inputs stream m-chunk-major (x chunk 0 + wqk on the HWDGE queue,
    rest on the gpsimd cast-DMA queue); the prefix computes only
    K01(mc0), Q01(lc0), V(lt0-3) and attention starts ~15us in; all
    remaining QKV/V/projection work is emitted as fine-grained filler
    units in the attention loop's PE slack.
  - softmax: denominator rides in the 65th V column (ones); reciprocal
    on DVE; partition-broadcast via a DRAM bounce (2 DMA hops);
    projection lags one l-chunk.

All matmuls run in float32r (single-pass fp32 PE mode).
"""

import math
import sys

sys.path.insert(0, "/opt/trn_rl_repo")

import numpy as np

import concourse.bass as bass
import concourse.tile as tile
from concourse import bacc, bass_utils, mybir

B, L, DIM, H, D = 2, 2048, 1024, 16, 64
N_CORES = 8
HL = 4  # heads per core
F = HL * D  # 256: per-core head feature width
LC, LT, CT = 512, 128, 128  # l-chunk, l/m-tile, contraction tile
N_LC, N_LT, N_CT = L // LC, L // LT, DIM // CT

DT = mybir.dt.float32r
F32 = mybir.dt.float32

_build_cache = {}


def _build(with_mask: bool):
    if with_mask in _build_cache:
        return _build_cache[with_mask]

    nc = bacc.Bacc("TRN2", target_bir_lowering=False, debug=False, num_devices=N_CORES)
    xT = nc.dram_tensor("xT", [DIM, L], F32, kind="ExternalInput").ap()
    wqk = nc.dram_tensor("wqk", [DIM, 2 * F], F32, kind="ExternalInput").ap()
    wv = nc.dram_tensor("wv", [DIM, F], F32, kind="ExternalInput").ap()
    wp = nc.dram_tensor("wp", [F, DIM], F32, kind="ExternalInput").ap()
    if with_mask:
        maskT = nc.dram_tensor("maskT", [HL, L, L], F32, kind="ExternalInput").ap()
    y = nc.dram_tensor("y", [L, DIM], F32, kind="ExternalOutput").ap()

    Exp = mybir.ActivationFunctionType.Exp

    with tile.TileContext(nc) as tc:
        with (
            tc.tile_pool(name="consts", bufs=1) as consts,
            tc.tile_pool(name="work", bufs=3) as work,
            tc.tile_pool(name="drp", bufs=4, space="DRAM") as drp,
            tc.tile_pool(name="ps_s", bufs=2, space="PSUM") as ps_s_pool,
            tc.tile_pool(name="ps_o", bufs=2, space="PSUM") as ps_o_pool,
            tc.tile_pool(name="ps_mm", bufs=2, space="PSUM") as ps_mm_pool,
        ):
            # ---- PE warmup: dummy matmuls during input DMA so the HAM
            # clock-gate reaches 2.4 GHz (64-row mode, like everything) ----
            warm = consts.tile([128, 512], mybir.dt.bfloat16)
            nc.vector.memset(warm, 0.0)
            ps_w = ps_mm_pool.tile([128, 512], F32, name="ps_w", tag="mm")
            for i in range(40):
                nc.tensor.matmul(
                    ps_w, lhsT=warm[:, 0:128], rhs=warm, start=(i == 0), stop=(i == 39)
                )

            # ---- SBUF residents ----
            xT_sb = consts.tile([128, N_CT, L], DT)
            wqk_sb = consts.tile([128, N_CT, 2 * F], DT)
            wv_sb = consts.tile([128, N_CT, F], DT)
            wp_sb = consts.tile([128, 2, DIM], DT)
            qT_sb = consts.tile([128, 2, L], DT)  # packed: 0=Q01 1=Q23
            kT_sb = consts.tile([128, HL, L], DT)  # per head, zero-padded
            v_sb = consts.tile([128, N_LT, HL, D + 1], DT)
            oT_sb = consts.tile([128, 2, L], DT)

            nc.vector.memset(kT_sb.rearrange("p a b -> p (a b)").bitcast(F32), 0.0)

            ones_f32 = consts.tile([128, 64], F32)
            nc.vector.memset(ones_f32, 1.0)
            nc.vector.tensor_copy(
                v_sb[:, :, :, D : D + 1],
                ones_f32.rearrange("p (a b c) -> p a b c", a=N_LT, b=HL),
            )

            # ---- input DMA. sync/HWDGE queue: wqk + x chunk 0 (staged f32,
            # DVE cast to f32r). gpsimd/SWDGE cast-DMA queue: wv, x chunks
            # 1-3, wp. Queues run concurrently; HBM bandwidth is shared, so
            # urgent first. ----
            def load_rounded(dst_ap, src_ap, shape):
                stg = work.tile(list(shape), F32, name="stg", tag="stg", bufs=2)
                nc.sync.dma_start(out=stg, in_=src_ap)
                nc.vector.tensor_copy(dst_ap, stg)

            for c in range(0, N_CT, 2):
                srcw = bass.AP(
                    tensor=wqk.tensor,
                    offset=c * 128 * 2 * F,
                    ap=[[2 * F, 128], [128 * 2 * F, 2], [1, 2 * F]],
                )
                load_rounded(wqk_sb[:, c : c + 2, :], srcw, [128, 2, 2 * F])
            for c in range(0, N_CT, 2):
                srcx = bass.AP(
                    tensor=xT.tensor,
                    offset=c * 128 * L,
                    ap=[[L, 128], [128 * L, 2], [1, LC]],
                )
                load_rounded(xT_sb[:, c : c + 2, 0:LC], srcx, [128, 2, LC])

            srcv = bass.AP(
                tensor=wv.tensor,
                offset=0,
                ap=[[F, 128], [128 * F, N_CT], [1, F]],
            )
            nc.gpsimd.dma_start(out=wv_sb, in_=srcv)
            for mc in range(1, N_LC):
                for c in range(0, N_CT, 4):
                    srcx = bass.AP(
                        tensor=xT.tensor,
                        offset=c * 128 * L + mc * LC,
                        ap=[[L, 128], [128 * L, 4], [1, LC]],
                    )
                    nc.gpsimd.dma_start(
                        out=xT_sb[:, c : c + 4, mc * LC : (mc + 1) * LC], in_=srcx
                    )
            srcp = bass.AP(
                tensor=wp.tensor,
                offset=0,
                ap=[[DIM, 128], [128 * DIM, 2], [1, DIM]],
            )
            nc.gpsimd.dma_start(out=wp_sb, in_=srcp)

            # ---- producer groups, emitted whole (prefix) or as 2 half
            # units of 4 c-tiles each (fillers) ----
            def qk_group(ft, lc, half=None):
                lsl = slice(lc * LC, (lc + 1) * LC)
                if half in (None, 0):
                    ps = ps_mm_pool.tile([128, LC], F32, name="ps_qk", tag="mm")
                    qk_group.open_ps = ps
                else:
                    ps = qk_group.open_ps
                cs = range(N_CT) if half is None else range(half * 4, half * 4 + 4)
                for c in cs:
                    nc.tensor.matmul(
                        ps,
                        lhsT=wqk_sb[:, c, ft * 128 : (ft + 1) * 128],
                        rhs=xT_sb[:, c, lsl],
                        start=(c == 0),
                        stop=(c == N_CT - 1),
                    )
                if half in (None, 1):
                    if ft < 2:
                        nc.vector.tensor_copy(qT_sb[:, ft, lsl], ps)
                    else:
                        hp = ft - 2
                        nc.vector.tensor_copy(kT_sb[0:64, 2 * hp, lsl], ps[0:64, :])
                        nc.vector.tensor_copy(
                            kT_sb[64:128, 2 * hp + 1, lsl], ps[64:128, :]
                        )

            def v_group(lt):
                ps = ps_mm_pool.tile([128, F], F32, name="ps_v", tag="mm")
                for c in range(N_CT):
                    nc.tensor.matmul(
                        ps,
                        lhsT=xT_sb[:, c, lt * 128 : (lt + 1) * 128],
                        rhs=wv_sb[:, c, :],
                        start=(c == 0),
                        stop=(c == N_CT - 1),
                    )
                nc.vector.tensor_copy(
                    v_sb[:, lt, :, 0:D], ps.rearrange("p (h d) -> p h d", h=HL)
                )

            def project_group(lt, oc):
                osl = slice(oc * 512, (oc + 1) * 512)
                ps = ps_mm_pool.tile([128, 512], F32, name="ps_y", tag="mm")
                for ft in range(2):
                    nc.tensor.matmul(
                        ps,
                        lhsT=oT_sb[:, ft, lt * 128 : (lt + 1) * 128],
                        rhs=wp_sb[:, ft, osl],
                        start=(ft == 0),
                        stop=(ft == 1),
                    )
                yb = work.tile([128, 512], F32, name="yb", tag="yb", bufs=3)
                nc.vector.tensor_copy(yb, ps)
                nc.sync.dma_start(out=y[lt * 128 : (lt + 1) * 128, osl], in_=yb)

            # ---- filler queue: PE work interleaved into attention slack ----
            fillers = []

            def pop_fillers(n):
                for _ in range(n):
                    if not fillers:
                        return
                    fillers.pop(0)()

            # ---- prefix: just enough for attention (lc0, hp0) to start ----
            qk_group(2, 0)  # K01(mc0)
            qk_group(0, 0)  # Q01(lc0)
            for lt in range(4):
                v_group(lt)

            def halves(ft, lc):
                return [
                    (lambda f=ft, l=lc: qk_group(f, l, half=0)),
                    (lambda f=ft, l=lc: qk_group(f, l, half=1)),
                ]

            fillers += [lambda: v_group(4), lambda: v_group(5)]
            fillers += halves(2, 1)  # K01(mc1)
            fillers += [lambda: v_group(6), lambda: v_group(7)]
            fillers += halves(3, 0)  # K23(mc0)
            fillers += [lambda: v_group(8), lambda: v_group(9)]
            fillers += halves(2, 2)  # K01(mc2)
            fillers += [lambda: v_group(10), lambda: v_group(11)]
            fillers += halves(1, 0)  # Q23(lc0)
            fillers += halves(2, 3)  # K01(mc3)
            fillers += [lambda: v_group(12), lambda: v_group(13)]
            fillers += halves(3, 1)  # K23(mc1)
            fillers += [lambda: v_group(14), lambda: v_group(15)]

            later = []
            later += halves(3, 2) + halves(3, 3)  # K23(mc2,3) before pass 2
            later += halves(0, 1)  # Q01(lc1) before pass 3
            later += halves(1, 1)  # Q23(lc1) before pass 4

            # ---- attention passes ----
            def attention_pass(lc, hp, rate):
                lsl = slice(lc * LC, (lc + 1) * LC)
                po = [
                    ps_o_pool.tile([128, LC], F32, name="po", tag="o")
                    for _ in range(2)
                ]
                ps_s_q = []
                pt_q = []

                def s_pair(mt):
                    m0 = mt * 128
                    ps_s = ps_s_pool.tile([128, 2 * LC], F32, name="ps_s", tag="s")
                    for hh in range(2):
                        csl = slice(hh * LC, (hh + 1) * LC)
                        nc.tensor.matmul(
                            ps_s[:, csl],
                            lhsT=kT_sb[:, 2 * hp + hh, m0 : m0 + 128],
                            rhs=qT_sb[:, hp, lsl],
                            start=True,
                            stop=True,
                        )
                    if with_mask:
                        for hh in range(2):
                            h = 2 * hp + hh
                            mk = work.tile([128, LC], F32, name="mk", tag="mk", bufs=4)
                            nc.sync.dma_start(
                                out=mk, in_=maskT[h, m0 : m0 + 128, lsl]
                            )
                            nc.vector.tensor_add(
                                ps_s[:, hh * LC : (hh + 1) * LC],
                                ps_s[:, hh * LC : (hh + 1) * LC],
                                mk,
                            )
                    ps_s_q.append(ps_s)

                def do_exp():
                    ps_s = ps_s_q.pop(0)
                    pt = work.tile([128, 2 * LC], DT, name="pt", tag="pt", bufs=2)
                    nc.scalar.activation(pt, ps_s, Exp)
                    pt_q.append(pt)

                def av(mt):
                    pt = pt_q.pop(0)
                    for hh in range(2):
                        h = 2 * hp + hh
                        nc.tensor.matmul(
                            po[hh][0 : D + 1, :],
                            lhsT=v_sb[:, mt, h, :],
                            rhs=pt[:, hh * LC : (hh + 1) * LC],
                            start=(mt == 0),
                            stop=(mt == N_LT - 1),
                        )

                s_pair(0)
                for mt in range(N_LT):
                    if mt + 1 < N_LT:
                        s_pair(mt + 1)
                    do_exp()
                    av(mt)
                    pop_fillers(rate)

                # normalize: denominator row -> reciprocal -> partition
                # broadcast via one DRAM bounce -> multiply
                for hh in range(2):
                    off = 64 * hh
                    dn = work.tile([128, LC], F32, name="dn", tag="dn", bufs=2)
                    nc.vector.tensor_copy(dn[0 : D + 1, :], po[hh][0 : D + 1, :])
                    nc.vector.reciprocal(dn[D : D + 1, :], dn[D : D + 1, :])
                    dr = drp.tile([1, LC], F32, name="dr", tag="dr")
                    nc.scalar.dma_start(out=dr, in_=dn[D : D + 1, :])
                    rb = work.tile([64, LC], F32, name="rb", tag="rb", bufs=2)
                    bcast = bass.AP(
                        tensor=dr.tensor, offset=dr.offset, ap=[[0, 64], [1, LC]]
                    )
                    nc.scalar.dma_start(out=rb, in_=bcast)
                    nc.vector.tensor_mul(
                        oT_sb[off : off + 64, hp, lsl], dn[0:D, :], rb
                    )

            def proj_fillers(lc):
                return [
                    (lambda lt=lt_, oc=oc_: project_group(lt, oc))
                    for lt_ in range(lc * LC // 128, (lc + 1) * LC // 128)
                    for oc_ in range(2)
                ]

            for lc in range(N_LC):
                for hp in range(2):
                    rate = 2 if (lc == 0 and hp == 0) else 1
                    attention_pass(lc, hp, rate)
                    if lc == 0 and hp == 0:
                        fillers.extend(later)
                if lc == 0:
                    fillers += halves(0, 2) + halves(1, 2)  # Q(lc2)
                elif lc == 1:
                    fillers += halves(0, 3) + halves(1, 3)  # Q(lc3)
                    fillers += proj_fillers(0)
                elif lc == 2:
                    fillers += proj_fillers(1) + proj_fillers(2)
            # drain: remaining fillers + last chunk's projection
            pop_fillers(len(fillers))
            for lt_ in range(3 * LC // 128, L // 128):
                for oc_ in range(2):
                    project_group(lt_, oc_)

    nc.compile()
    _build_cache[with_mask] = nc
    return nc


def _prepare_in_maps(x, attn_mask, qkv_w, proj_w, s, with_mask):
    qk_scale = D ** -0.5
    q_scale = qk_scale * float(s) * math.log(L)
    x = np.asarray(x, np.float32)
    qkv_w = np.asarray(qkv_w, np.float32)
    proj_w = np.asarray(proj_w, np.float32)

    in_maps = []
    for core in range(N_CORES):
        b = core // (N_CORES // B)
        h0 = (core % (N_CORES // B)) * HL
        fs = slice(h0 * D, h0 * D + F)
        wq = qkv_w[0 * DIM : 1 * DIM][fs] * q_scale  # [F, DIM]
        wk = qkv_w[1 * DIM : 2 * DIM][fs]
        wvm = qkv_w[2 * DIM : 3 * DIM][fs]
        m = {
            "xT": np.ascontiguousarray(x[b].T),
            "wqk": np.ascontiguousarray(np.concatenate([wq, wk], axis=0).T),
            "wv": np.ascontiguousarray(wvm.T),
            "wp": np.ascontiguousarray(proj_w[:, fs].T),
        }
        if with_mask:
            m["maskT"] = np.ascontiguousarray(
                np.transpose(attn_mask[b, h0 : h0 + HL], (0, 2, 1))
            ).astype(np.float32)
        in_maps.append(m)
    return in_maps


def _postprocess(results, proj_b):
    gpb = N_CORES // B
    y = np.zeros((B, L, DIM), np.float32)
    for core in range(N_CORES):
        y[core // gpb] += results[core]["y"]
    y += np.asarray(proj_b, np.float32)[None, None, :]
    return y


def run(x, attn_mask, qkv_w, proj_w, proj_b, s, **spmd_kwargs):
    with_mask = bool(np.any(attn_mask))
    nc = _build(with_mask)
    in_maps = _prepare_in_maps(x, attn_mask, qkv_w, proj_w, s, with_mask)
    res = bass_utils.run_bass_kernel_spmd(
        nc, in_maps, core_ids=list(range(N_CORES)), **spmd_kwargs
    )
    return _postprocess(res.results, proj_b), res


def kernel(x, attn_mask, qkv_w, proj_w, proj_b, s):
    y, _ = run(x, attn_mask, qkv_w, proj_w, proj_b, s)
    return y
